# revision 45
# baseline (speedup 1.0000x reference)
"""Trainium2 Bass kernel for nn_Attention_56736517980393.

Reference computation (B=4, S=2048, C=1024, H=16 heads, D=64, MLP hidden 1024):
    q = (x @ Wq + bq) * D**-0.5          per-head [B,H,S,D]
    k = data @ Wk + bk ; v = data @ Wv + bv
    kv[b,h] = k^T @ v                     [D,D]   (no softmax -> associative form)
    attn = q @ kv                         [B,S,C]
    out = x + attn + gelu(attn @ W1 + b1) @ W2 + b2

Sharding: 8 cores = (batch b in 0..3) x (sequence half j in 0..1).
Each core computes K/V projections for its (b, j) sequence half, a partial
kv (reduced over its half), AllReduces kv with its pair core, then computes
attention / MLP / residual for its half.  Activations are kept feature-major
(transposed, [C, S]) on chip so biases are per-partition and no on-chip
transposes are needed.

Design notes:
  - Everything streams f16 (inputs, weights, intermediates, output);
    rel err ~8e-4 vs the fp32 reference.
  - kv is accumulated as 8 head-PAIR [128,128] blocks; the off-diagonal
    64x64 quadrants are cross-head garbage discarded at eviction.
    Attention then uses a BLOCK-DIAGONAL kv operand (kvbd block m = kv_2m
    top-left, kv_2m+1 bottom-right, zeros off-diagonal) so one [128,512]
    matmul yields both heads: attn phase is 8192 PE cycles total.
  - Phase-1 runs nb-major (all st for feature half 0, then half 1) so the
    kv AllReduce splits in two: pairs 0-3 reduce while the nb=1 sweep
    (~30us of PE work) runs, pairs 4-7 reduce under the Q nb=1 sweep.
  - Q-projection nb=0 groups are interleaved INTO the phase-1 nb=0 sweep
    (they only need xT/wq), attention nb=0 into the Q nb=1 sweep, and
    attention nb=1 into the MLP1 nb=0 sweep: the PE stream never pauses
    for an eviction or the collective (the cost of any PE gap is a
    clock-ramp reset, ~2us).
  - Each operand loads with a few strided mega-DMAs (>=512B contiguous
    runs) ordered by first consumption; wq loads per-m-slice so the first
    Q group can start ~1.5us after xT half 0 lands.
"""

import numpy as np

B, S, C, H, D = 4, 2048, 1024, 16, 64
SH = S // 2          # sequence rows per core
SCALE = D ** -0.5
P = 128              # SBUF partitions
NB = 512             # matmul moving free-dim block (one PSUM bank of fp32)
KT = C // P          # 8 contraction tiles
MT = C // P          # 8 output-feature tiles
NBLK = SH // NB      # 2 sequence blocks
HP = H // 2          # 8 head pairs

# how many Q nb=0 groups to emit after each phase-1 nb=0 st-iteration
Q_SCHED = [0, 0, 1, 1, 2, 1, 2, 1]

_CACHE = {}


def _build(with_bkv: bool, loop_r: int = 1, n_cores: int = 8, use_cc: bool = True,
           phases: int = 4):
    import concourse.bacc as bacc
    import concourse.mybir as mybir
    from concourse.tile import TileContext

    F32 = mybir.dt.float32
    F16 = mybir.dt.float16
    AF = mybir.ActivationFunctionType
    ALU = mybir.AluOpType

    nc = bacc.Bacc(
        trn_type="TRN2", target_bir_lowering=False, debug=False, num_devices=n_cores
    )

    xT = nc.dram_tensor("xT", [C, SH], F16, kind="ExternalInput").ap()
    dT = nc.dram_tensor("dT", [C, SH], F16, kind="ExternalInput").ap()
    wq = nc.dram_tensor("wq", [C, C], F16, kind="ExternalInput").ap()
    wk = nc.dram_tensor("wk", [C, C], F16, kind="ExternalInput").ap()
    wv = nc.dram_tensor("wv", [C, C], F16, kind="ExternalInput").ap()
    w1 = nc.dram_tensor("w1", [C, C], F16, kind="ExternalInput").ap()
    w2 = nc.dram_tensor("w2", [C, C], F16, kind="ExternalInput").ap()
    # feature-major biases: [128, 8]; column m is the per-partition bias for
    # feature tile m
    bqt = nc.dram_tensor("bqt", [P, MT], F32, kind="ExternalInput").ap()
    b1t = nc.dram_tensor("b1t", [P, MT], F32, kind="ExternalInput").ap()
    b2t = nc.dram_tensor("b2t", [P, MT], F32, kind="ExternalInput").ap()
    if with_bkv:
        bkr = nc.dram_tensor("bkr", [1, C], F16, kind="ExternalInput").ap()
        bvr = nc.dram_tensor("bvr", [1, C], F16, kind="ExternalInput").ap()
    outT = nc.dram_tensor("outT", [C, SH], F16, kind="ExternalOutput").ap()

    groups = [[i, i + 1] for i in range(0, n_cores, 2)]

    def dram3(t, c0, c1):
        """DRAM [C, cols] tensor viewed as [128, KT, c1-c0]: row-tile kt's
        columns c0:c1 land at (partition p, index kt)."""
        return t.rearrange("(k p) s -> p k s", p=P)[:, :, c0:c1]

    with TileContext(nc) as tc:
        with tc.tile_pool(name="sb", bufs=1) as sb, \
             tc.tile_pool(name="ps", bufs=1, space="PSUM") as psp, \
             tc.tile_pool(name="dr", bufs=1, space="DRAM") as drp:

            def body(it=None):
                # ---- big per-operand tiles; [:, kt*W + c] = tile kt col c ----
                def big(name, w=C):
                    return sb.tile([P, KT * w], F16, tag=name, bufs=1, name=name)

                dTa, xTa = big("dTa", SH), big("xTa", SH)
                wka, wva, wqa, w1a, w2a = (big(n) for n in
                                           ("wka", "wva", "wqa", "w1a", "w2a"))

                def load(tile, dram, c0, c1, k0=0, k1=KT):
                    out = tile[:].rearrange("p (k s) -> p k s", k=KT)[:, k0:k1, c0:c1]
                    in_ = dram.rearrange("(k p) s -> p k s", p=P)[:, k0:k1, c0:c1]
                    nc.sync.dma_start(out=out, in_=in_)

                bq_sb = sb.tile([P, MT], F32, tag="bias", bufs=3, name="bq_sb")
                b1_sb = sb.tile([P, MT], F32, tag="bias", bufs=3, name="b1_sb")
                b2_sb = sb.tile([P, MT], F32, tag="bias", bufs=3, name="b2_sb")

                # DMA issue order ~ first-consumption order of the PE stream;
                # the first chunks are kt-split so the first K-proj matmuls can
                # start while the rest of the half streams in.
                load(dTa, dT, 0, 256, 0, 4)      # st 0-1, kt 0-3
                load(wka, wk, 0, NB, 0, 4)
                load(dTa, dT, 0, 256, 4, 6)      # st 0-1, kt 4-5
                load(wka, wk, 0, NB, 4, 6)
                load(dTa, dT, 0, 256, 6, 8)      # st 0-1, kt 6-7
                load(wka, wk, 0, NB, 6, 8)
                load(wva, wv, 0, NB, 0, 4)
                load(wva, wv, 0, NB, 4, 6)
                load(wva, wv, 0, NB, 6, 8)
                load(dTa, dT, 256, 512)          # st 2-3
                load(dTa, dT, 512, 768)          # st 4-5
                load(xTa, xT, 0, NB)
                load(wqa, wq, 0, P)              # wq m-slice 0
                nc.sync.dma_start(out=bq_sb[:], in_=bqt[:])
                load(wqa, wq, P, 2 * P)          # wq m-slice 1
                load(dTa, dT, 768, SH)           # st 6-7
                for mm_ in range(2, MT):         # wq m-slices 2-7
                    load(wqa, wq, mm_ * P, (mm_ + 1) * P)
                load(wka, wk, NB, C)
                load(wva, wv, NB, C)
                load(xTa, xT, NB, SH)
                load(w1a, w1, 0, NB)
                load(w1a, w1, NB, C)
                load(w2a, w2, 0, NB)
                load(w2a, w2, NB, C)
                nc.sync.dma_start(out=b1_sb[:], in_=b1t[:])
                nc.sync.dma_start(out=b2_sb[:], in_=b2t[:])
                if with_bkv:
                    bk_sb = sb.tile([1, C], F16, tag="brow", bufs=2, name="bk_sb")
                    bv_sb = sb.tile([1, C], F16, tag="brow", bufs=2, name="bv_sb")
                    ones = sb.tile([1, P], F16, tag="ones", bufs=1, name="ones")
                    nc.sync.dma_start(out=bk_sb[:], in_=bkr[:])
                    nc.sync.dma_start(out=bv_sb[:], in_=bvr[:])
                    nc.vector.memset(ones[:], 1.0)

                # PE warmup: dummy matmuls during the initial DMA wait so the
                # clock ramp (and HAM window) completes before the first real
                # matmul group arrives; fine-grained (N=128) so the filler
                # ends right as the first weight chunk lands (~5.2us).
                scr = sb.tile([64, 64 + P], F16, tag="scr", bufs=1, name="scr")
                nc.vector.memset(scr[:], 0.0)

                def warm(n):
                    for _ in range(n):
                        wp = psp.tile([64, P], F32, tag="mm", bufs=6, name="wp")
                        nc.tensor.matmul(wp[:], lhsT=scr[:, 0:64],
                                         rhs=scr[:, 64:64 + P],
                                         start=True, stop=True)

                if it in (None, 0):
                    # only the first unrolled body needs the ramp warmup; in
                    # steady state the previous body keeps the PE busy
                    warm(48)

                # block-diagonal kv operand; zeroed early, off the critical path
                kvbd = sb.tile([P, MT * P], F16, tag="kvr", bufs=1, name="kvbd")
                nc.vector.memset(kvbd[:], 0.0)

                # ---- Q projection groups: qt_all[:, g*NB:], g = nb*MT + m ----
                qt_all = sb.tile([P, 2 * MT * NB], F16, tag="qt", bufs=1,
                                 name="qt_all")

                def q_group(m, nb):
                    g = nb * MT + m
                    qp = psp.tile([P, NB], F32, tag="mm", bufs=6, name="qp")
                    for kt in range(KT):
                        nc.tensor.matmul(
                            qp[:],
                            lhsT=wqa[:, kt * C + m * P:kt * C + (m + 1) * P],
                            rhs=xTa[:, kt * SH + nb * NB:kt * SH + (nb + 1) * NB],
                            start=(kt == 0), stop=(kt == KT - 1),
                        )
                    nc.scalar.activation(qt_all[:, g * NB:(g + 1) * NB], qp[:],
                                         AF.Identity, bias=bq_sb[:, m:m + 1],
                                         scale=1.0)

                # ---- phase-1 helpers ----
                kv_ps_a = psp.tile([P, NB], F32, tag="acc", bufs=2, name="kv_ps_a")
                kv_ps_b = psp.tile([P, NB], F32, tag="acc", bufs=2, name="kv_ps_b")

                def proj_half(w_big, brow, st, nb, nm, fills=None):
                    t = sb.tile([P, NB], F16, tag="kvs", bufs=6, name=nm)
                    pk = psp.tile([P, NB], F32, tag="mm", bufs=6, name="pk")
                    for kt in range(KT):
                        if fills:
                            warm(fills.get(kt, 0))
                        nc.tensor.matmul(
                            pk[:],
                            lhsT=dTa[:, kt * SH + st * P:kt * SH + (st + 1) * P],
                            rhs=w_big[:, kt * C + nb * NB:kt * C + (nb + 1) * NB],
                            start=(kt == 0),
                            stop=(kt == KT - 1 and not with_bkv),
                        )
                    if with_bkv:
                        nc.tensor.matmul(
                            pk[:], lhsT=ones[:], rhs=brow[:, nb * NB:(nb + 1) * NB],
                            start=False, stop=True,
                        )
                    nc.scalar.copy(t[:], pk[:])
                    return t

                def kv_mms(k_h, v_h, st, half):
                    # head pairs half*4 .. half*4+3; pair tp at free offset
                    # (tp%4)*128 of bank a (tp<4) or b
                    for i in range(4):
                        tp = half * 4 + i
                        tgt = kv_ps_a if tp < 4 else kv_ps_b
                        fo = (tp % 4) * P
                        psl = slice(i * P, (i + 1) * P)
                        # start/stop once per PSUM bank (zero region), not
                        # per pair block
                        nc.tensor.matmul(
                            tgt[:, fo:fo + P],
                            lhsT=k_h[:, psl],
                            rhs=v_h[:, psl],
                            start=(st == 0 and i == 0),
                            stop=(st == MT - 1 and i == 3),
                        )

                kv_pend = []

                def p1_iter(st, nb, n_q, kf=None, vf=None):
                    # K group, then a lagged kv burst (2 iterations behind so
                    # the k/v evictions are never on the PE critical path),
                    # then V group, then any scheduled Q groups
                    k_h = proj_half(wka, bk_sb if with_bkv else None, st, nb,
                                    "k_h", kf)
                    if len(kv_pend) >= 2:
                        kv_mms(*kv_pend.pop(0))
                    v_h = proj_half(wva, bv_sb if with_bkv else None, st, nb,
                                    "v_h", vf)
                    kv_pend.append((k_h, v_h, st, nb))
                    for _ in range(n_q):
                        q_group(next(qiter), 0)

                qiter = iter(range(MT))

                # ---- kv eviction (diagonal quadrants) + pairwise AllReduce,
                # split per bank so each half reduces under ~30us of PE work --
                def kv_reduce(half, ps):
                    kv_s = sb.tile([P, NB // 2], F16, tag="kvio", bufs=2,
                                   name=f"kv_s{half}")
                    for r0 in (0, 64):
                        src = ps[r0:r0 + 64, :].rearrange(
                            "p (t x) -> p t x", x=P)[:, :, r0:r0 + 64]
                        dst = kv_s[r0:r0 + 64, :].rearrange(
                            "p (t x) -> p t x", x=64)
                        if r0 == 0:
                            nc.vector.tensor_copy(dst, src)
                        else:
                            nc.scalar.copy(dst, src)
                    kv_i = drp.tile([P, NB // 2], F16, tag="dri", bufs=2,
                                    name=f"kv_i{half}")
                    kv_o = drp.tile([P, NB // 2], F16, tag="dro", bufs=2,
                                    name=f"kv_o{half}")
                    nc.sync.dma_start(out=kv_i[:], in_=kv_s[:])
                    if n_cores == 1 or not use_cc:
                        # single-core analysis build: stand in for the AllReduce
                        nc.sync.dma_start(out=kv_o[:], in_=kv_i[:])
                    else:
                        nc.gpsimd.collective_compute(
                            "AllReduce", ALU.add, replica_groups=groups,
                            ins=[kv_i.opt()], outs=[kv_o.opt()],
                        )
                    # scatter diagonals into kvbd blocks half*4 .. half*4+3
                    for r0 in (0, 64):
                        dst = kvbd[r0:r0 + 64, half * 4 * P:(half + 1) * 4 * P
                                   ].rearrange("p (t x) -> p t x", x=P)[:, :,
                                                                        r0:r0 + 64]
                        src = kv_o[r0:r0 + 64, :].rearrange(
                            "p (t x) -> p t x", x=64)
                        nc.sync.dma_start(out=dst, in_=src)

                # ---- attention: one block-diagonal matmul per (m, nb) ----
                attn_sb = [sb.tile([P, SH], F16, tag="at", bufs=8, name=f"attn{m}")
                           for m in range(MT)]

                def attn_mm(m, nb):
                    g = nb * MT + m
                    nsl = slice(nb * NB, (nb + 1) * NB)
                    ap = psp.tile([P, NB], F32, tag="mm", bufs=6, name="ap")
                    nc.tensor.matmul(
                        ap[:],
                        lhsT=kvbd[:, m * P:(m + 1) * P],
                        rhs=qt_all[:, g * NB:(g + 1) * NB],
                        start=True, stop=True)
                    if m % 2 == 0:
                        nc.vector.tensor_copy(attn_sb[m][:, nsl], ap[:])
                    else:
                        nc.scalar.copy(attn_sb[m][:, nsl], ap[:])

                def mlp1_group(m, nb):
                    nsl = slice(nb * NB, (nb + 1) * NB)
                    hp = psp.tile([P, NB], F32, tag="mm", bufs=6, name="hp")
                    for kt in range(KT):
                        nc.tensor.matmul(
                            hp[:],
                            lhsT=w1a[:, kt * C + m * P:kt * C + (m + 1) * P],
                            rhs=attn_sb[kt][:, nsl],
                            start=(kt == 0), stop=(kt == KT - 1),
                        )
                    nc.scalar.activation(h1_sb[m][:, nsl], hp[:], AF.Gelu,
                                         bias=b1_sb[:, m:m + 1], scale=1.0)

                # ================= PE stream =================
                for st in range(MT):             # phase-1 nb=0 + Q nb=0 groups
                    p1_iter(st, 0, Q_SCHED[st])
                for m in qiter:                  # any Q groups Q_SCHED missed
                    q_group(m, 0)
                for st in range(MT):             # phase-1 nb=1
                    p1_iter(st, 1, 0)
                    if st == 1:                  # nb=0 kv bursts fully drained
                        kv_reduce(0, kv_ps_a)    # AR pairs 0-3 under nb=1 sweep

                if phases <= 1:
                    while kv_pend:
                        kv_mms(*kv_pend.pop(0))
                    kv_reduce(1, kv_ps_b)
                    # dump kvbd [128, 1024] into the first SH columns of outT
                    stg0 = sb.tile([P, MT * P], F16, tag="stgd", bufs=1, name="stg0")
                    nc.vector.tensor_copy(stg0[:], kvbd[:])
                    for m in range(MT):
                        nc.sync.dma_start(out=outT[m * P:(m + 1) * P, 0:MT * P],
                                          in_=stg0[:])
                    return

                # Q nb=1 sweep: drain the last kv bursts + kick AR-b early,
                # then attention nb=0 interleaved (m 0-3 need only AR half a;
                # m 4-7 sit as late as possible to maximize AR-b slack)
                attn0_sched = {2: [0], 3: [1], 4: [2], 5: [3],
                               6: [4], 7: [5, 6]}
                for m in range(MT):
                    q_group(m, 1)
                    while kv_pend:
                        kv_mms(*kv_pend.pop(0))
                        if not kv_pend:
                            kv_reduce(1, kv_ps_b)
                    for am in attn0_sched.get(m, []):
                        attn_mm(am, 0)

                if phases <= 2:
                    # only attn nb=0 exists at this phase cut
                    for m in range(MT):
                        stg = sb.tile([P, NB], F16, tag="stg", bufs=3, name="stg")
                        nc.vector.tensor_copy(stg[:], attn_sb[m][:, 0:NB])
                        nc.sync.dma_start(out=outT[m * P:(m + 1) * P, 0:NB],
                                          in_=stg[:])
                    return

                # MLP1 nb=0 sweep with attention nb=1 interleaved; the last
                # attn nb=0 matmul (needs AR-b) goes first, right before the
                # MLP1 groups that consume it
                h1_sb = [sb.tile([P, SH], F16, tag="h1", bufs=8, name=f"h1{m}")
                         for m in range(MT)]
                attn_mm(0, 1)
                attn_mm(7, 0)
                attn_mm(1, 1)
                for m in range(MT):
                    mlp1_group(m, 0)
                    if m + 2 < MT:
                        attn_mm(m + 2, 1)
                for m in range(MT):
                    mlp1_group(m, 1)

                if phases <= 3:
                    for m in range(MT):
                        stg = sb.tile([P, SH], F16, tag="stg", bufs=3, name="stg")
                        nc.vector.tensor_copy(stg[:], h1_sb[m][:])
                        nc.sync.dma_start(out=outT[m * P:(m + 1) * P, :],
                                          in_=stg[:])
                    return

                # residual x + attn (f16; rel contribution ~2e-4)
                xacc = [sb.tile([P, SH], F16, tag="xr", bufs=8, name=f"xacc{m}")
                        for m in range(MT)]
                for m in range(MT):
                    nc.vector.tensor_add(
                        xacc[m][:],
                        xTa[:].rearrange("p (k s) -> p k s", k=KT)[:, m, :],
                        attn_sb[m][:])

                # ---- MLP out + residuals (m-outer; output DMA per half) ----
                for m in range(MT):
                    stg = sb.tile([P, SH], F16, tag="stg", bufs=3, name="stg")
                    for nb in range(NBLK):
                        nsl = slice(nb * NB, (nb + 1) * NB)
                        # the very last group runs as two N=256 psum groups so
                        # the final eviction+store chain is half as long
                        subs = ([(0, 256), (256, NB)]
                                if (m == MT - 1 and nb == NBLK - 1) else
                                [(0, NB)])
                        for s0, s1 in subs:
                            ssl = slice(nb * NB + s0, nb * NB + s1)
                            op = psp.tile([P, s1 - s0], F32, tag="mm", bufs=6,
                                          name="op")
                            for kt in range(KT):
                                nc.tensor.matmul(
                                    op[:],
                                    lhsT=w2a[:, kt * C + m * P:kt * C + (m + 1) * P],
                                    rhs=h1_sb[kt][:, ssl],
                                    start=(kt == 0), stop=(kt == KT - 1),
                                )
                            # stg = (op + b2) + (x + attn)
                            nc.vector.scalar_tensor_tensor(
                                stg[:, ssl], op[:], b2_sb[:, m:m + 1],
                                xacc[m][:, ssl], op0=ALU.add, op1=ALU.add)
                            nc.sync.dma_start(out=outT[m * P:(m + 1) * P, ssl],
                                              in_=stg[:, ssl])

            # Straight-line unroll for timing runs (collectives cannot sit
            # inside a hardware For_i loop on this execution path).
            for it_ in range(loop_r):
                body(it_)

    nc.compile()
    return nc


def _get_program(with_bkv: bool, loop_r: int = 1, use_cc: bool = True,
                 phases: int = 4):
    key = (with_bkv, loop_r, use_cc, phases)
    if key not in _CACHE:
        _CACHE[key] = _build(with_bkv, loop_r, use_cc=use_cc, phases=phases)
    return _CACHE[key]


def _pack_inputs(x, data, Wq, bq, Wk, bk, Wv, bv, W1, b1, W2, b2, with_bkv):
    f32, f16 = np.float32, np.float16
    wq_s = np.ascontiguousarray((np.asarray(Wq, f32) * f32(SCALE)).astype(f16))
    wk_c = np.ascontiguousarray(np.asarray(Wk, f32).astype(f16))
    wv_c = np.ascontiguousarray(np.asarray(Wv, f32).astype(f16))
    w1_c = np.ascontiguousarray(np.asarray(W1, f32).astype(f16))
    w2_c = np.ascontiguousarray(np.asarray(W2, f32).astype(f16))
    bqt = np.ascontiguousarray((np.asarray(bq, f32) * f32(SCALE)).reshape(MT, P).T)
    b1t = np.ascontiguousarray(np.asarray(b1, f32).reshape(MT, P).T)
    b2t = np.ascontiguousarray(np.asarray(b2, f32).reshape(MT, P).T)
    x = np.asarray(x, f32)
    data = np.asarray(data, f32)
    in_maps = []
    for c in range(8):
        b_, j = divmod(c, 2)
        m = {
            "xT": np.ascontiguousarray(x[b_, j * SH:(j + 1) * SH, :].T.astype(f16)),
            "dT": np.ascontiguousarray(data[b_, j * SH:(j + 1) * SH, :].T.astype(f16)),
            "wq": wq_s, "wk": wk_c, "wv": wv_c, "w1": w1_c, "w2": w2_c,
            "bqt": bqt, "b1t": b1t, "b2t": b2t,
        }
        if with_bkv:
            m["bkr"] = np.asarray(bk, f32).astype(f16).reshape(1, C)
            m["bvr"] = np.asarray(bv, f32).astype(f16).reshape(1, C)
        in_maps.append(m)
    return in_maps


def run_on_hw(inputs, loop_r: int = 1, trace: bool = False):
    """Run the SPMD program; returns BassKernelResults."""
    from concourse.bass_utils import run_bass_kernel_spmd

    with_bkv = bool(
        np.any(np.asarray(inputs["bk"])) or np.any(np.asarray(inputs["bv"]))
    )
    nc = _get_program(with_bkv, loop_r)
    in_maps = _pack_inputs(
        inputs["x"], inputs["data"], inputs["Wq"], inputs["bq"], inputs["Wk"],
        inputs["bk"], inputs["Wv"], inputs["bv"], inputs["W1"], inputs["b1"],
        inputs["W2"], inputs["b2"], with_bkv,
    )
    res = run_bass_kernel_spmd(nc, in_maps, list(range(8)), trace=trace)
    return res


def kernel(**inputs) -> np.ndarray:
    res = run_on_hw(inputs, loop_r=1)
    out = np.empty((B, S, C), dtype=np.float32)
    for c in range(8):
        b_, j = divmod(c, 2)
        out[b_, j * SH:(j + 1) * SH, :] = res.results[c]["outT"].astype(np.float32).T
    return out


# revision 50
# speedup vs baseline: 1.1636x; 1.1636x over previous
"""Trainium2 Bass kernel for nn_Attention_56736517980393.

Reference computation (B=4, S=2048, C=1024, H=16 heads, D=64, MLP hidden 1024):
    q = (x @ Wq + bq) * D**-0.5          per-head [B,H,S,D]
    k = data @ Wk + bk ; v = data @ Wv + bv
    kv[b,h] = k^T @ v                     [D,D]   (no softmax -> associative form)
    attn = q @ kv                         [B,S,C]
    out = x + attn + gelu(attn @ W1 + b1) @ W2 + b2

Sharding: 8 cores = (batch b in 0..3) x (sequence half j in 0..1).
Each core computes K/V projections for its (b, j) sequence half, a partial
kv (reduced over its half), AllReduces kv with its pair core, then computes
attention / MLP / residual for its half.  Activations are kept feature-major
(transposed, [C, S]) on chip so biases are per-partition and no on-chip
transposes are needed.

Design notes:
  - Everything streams f16 (inputs, weights, intermediates, output);
    rel err ~8e-4 vs the fp32 reference.
  - kv is accumulated as 8 head-PAIR [128,128] blocks; the off-diagonal
    64x64 quadrants are cross-head garbage discarded at eviction.
    Attention then uses a BLOCK-DIAGONAL kv operand (kvbd block m = kv_2m
    top-left, kv_2m+1 bottom-right, zeros off-diagonal) so one [128,512]
    matmul yields both heads: attn phase is 8192 PE cycles total.
  - Phase-1 runs nb-major (all st for feature half 0, then half 1) so the
    kv AllReduce splits in two: pairs 0-3 reduce while the nb=1 sweep
    (~30us of PE work) runs, pairs 4-7 reduce under the Q nb=1 sweep.
  - Q-projection nb=0 groups are interleaved INTO the phase-1 nb=0 sweep
    (they only need xT/wq), attention nb=0 into the Q nb=1 sweep, and
    attention nb=1 into the MLP1 nb=0 sweep: the PE stream never pauses
    for an eviction or the collective (the cost of any PE gap is a
    clock-ramp reset, ~2us).
  - Each operand loads with a few strided mega-DMAs (>=512B contiguous
    runs) ordered by first consumption; wq loads per-m-slice so the first
    Q group can start ~1.5us after xT half 0 lands.
"""

import numpy as np

B, S, C, H, D = 4, 2048, 1024, 16, 64
SH = S // 2          # sequence rows per core
SCALE = D ** -0.5
P = 128              # SBUF partitions
NB = 512             # matmul moving free-dim block (one PSUM bank of fp32)
KT = C // P          # 8 contraction tiles
MT = C // P          # 8 output-feature tiles
NBLK = SH // NB      # 2 sequence blocks
HP = H // 2          # 8 head pairs

# how many Q nb=0 groups to emit after each phase-1 nb=0 st-iteration
Q_SCHED = [0, 0, 1, 1, 2, 1, 2, 1]

_CACHE = {}


def _build(with_bkv: bool, loop_r: int = 1, n_cores: int = 8, use_cc: bool = True,
           phases: int = 4):
    import concourse.bacc as bacc
    import concourse.mybir as mybir
    from concourse.tile import TileContext

    F32 = mybir.dt.float32
    F16 = mybir.dt.float16
    AF = mybir.ActivationFunctionType
    ALU = mybir.AluOpType

    nc = bacc.Bacc(
        trn_type="TRN2", target_bir_lowering=False, debug=False, num_devices=n_cores
    )

    xT = nc.dram_tensor("xT", [C, SH], F16, kind="ExternalInput").ap()
    dT = nc.dram_tensor("dT", [C, SH], F16, kind="ExternalInput").ap()
    wq = nc.dram_tensor("wq", [C, C], F16, kind="ExternalInput").ap()
    wk = nc.dram_tensor("wk", [C, C], F16, kind="ExternalInput").ap()
    wv = nc.dram_tensor("wv", [C, C], F16, kind="ExternalInput").ap()
    w1 = nc.dram_tensor("w1", [C, C], F16, kind="ExternalInput").ap()
    w2 = nc.dram_tensor("w2", [C, C], F16, kind="ExternalInput").ap()
    # feature-major biases: [128, 8]; column m is the per-partition bias for
    # feature tile m
    bqt = nc.dram_tensor("bqt", [P, MT], F32, kind="ExternalInput").ap()
    b1t = nc.dram_tensor("b1t", [P, MT], F32, kind="ExternalInput").ap()
    b2t = nc.dram_tensor("b2t", [P, MT], F32, kind="ExternalInput").ap()
    if with_bkv:
        bkr = nc.dram_tensor("bkr", [1, C], F16, kind="ExternalInput").ap()
        bvr = nc.dram_tensor("bvr", [1, C], F16, kind="ExternalInput").ap()
    outT = nc.dram_tensor("outT", [C, SH], F16, kind="ExternalOutput").ap()

    groups = [[i, i + 1] for i in range(0, n_cores, 2)]

    def dram3(t, c0, c1):
        """DRAM [C, cols] tensor viewed as [128, KT, c1-c0]: row-tile kt's
        columns c0:c1 land at (partition p, index kt)."""
        return t.rearrange("(k p) s -> p k s", p=P)[:, :, c0:c1]

    with TileContext(nc) as tc:
        with tc.tile_pool(name="sb", bufs=1) as sb, \
             tc.tile_pool(name="ps", bufs=1, space="PSUM") as psp, \
             tc.tile_pool(name="dr", bufs=1, space="DRAM") as drp:

            def body(it=None):
                # ---- big per-operand tiles; [:, kt*W + c] = tile kt col c ----
                def big(name, w=C):
                    return sb.tile([P, KT * w], F16, tag=name, bufs=1, name=name)

                dTa, xTa = big("dTa", SH), big("xTa", SH)
                wka, wva, wqa, w1a, w2a = (big(n) for n in
                                           ("wka", "wva", "wqa", "w1a", "w2a"))

                def load(tile, dram, c0, c1, k0=0, k1=KT, eng=None):
                    out = tile[:].rearrange("p (k s) -> p k s", k=KT)[:, k0:k1, c0:c1]
                    in_ = dram.rearrange("(k p) s -> p k s", p=P)[:, k0:k1, c0:c1]
                    (eng or nc.sync).dma_start(out=out, in_=in_)

                bq_sb = sb.tile([P, MT], F32, tag="bias", bufs=3, name="bq_sb")
                b1_sb = sb.tile([P, MT], F32, tag="bias", bufs=3, name="b1_sb")
                b2_sb = sb.tile([P, MT], F32, tag="bias", bufs=3, name="b2_sb")

                # DMA issue order ~ first-consumption order of the PE stream;
                # the first chunks are kt-split so the first K-proj matmuls can
                # start while the rest of the half streams in.
                load(dTa, dT, 0, 256, 0, 4)      # st 0-1, kt 0-3
                load(wka, wk, 0, NB, 0, 4)
                load(dTa, dT, 0, 256, 4, 6)      # st 0-1, kt 4-5
                load(wka, wk, 0, NB, 4, 6)
                load(dTa, dT, 0, 256, 6, 8)      # st 0-1, kt 6-7
                load(wka, wk, 0, NB, 6, 8)
                load(wva, wv, 0, NB, 0, 4)
                load(wva, wv, 0, NB, 4, 6)
                load(wva, wv, 0, NB, 6, 8)
                load(dTa, dT, 256, 512)          # st 2-3
                load(dTa, dT, 512, 768)          # st 4-5
                load(xTa, xT, 0, NB)
                load(wqa, wq, 0, P)              # wq m-slice 0
                nc.sync.dma_start(out=bq_sb[:], in_=bqt[:])
                load(wqa, wq, P, 2 * P)          # wq m-slice 1
                load(dTa, dT, 768, SH)           # st 6-7
                for mm_ in range(2, MT):         # wq m-slices 2-7
                    load(wqa, wq, mm_ * P, (mm_ + 1) * P)
                load(wka, wk, NB, C)
                load(wva, wv, NB, C)
                load(xTa, xT, NB, SH)
                load(w1a, w1, 0, NB)
                load(w1a, w1, NB, C)
                load(w2a, w2, 0, NB)
                load(w2a, w2, NB, C)
                nc.sync.dma_start(out=b1_sb[:], in_=b1t[:])
                nc.sync.dma_start(out=b2_sb[:], in_=b2t[:])
                if with_bkv:
                    bk_sb = sb.tile([1, C], F16, tag="brow", bufs=2, name="bk_sb")
                    bv_sb = sb.tile([1, C], F16, tag="brow", bufs=2, name="bv_sb")
                    ones = sb.tile([1, P], F16, tag="ones", bufs=1, name="ones")
                    nc.sync.dma_start(out=bk_sb[:], in_=bkr[:])
                    nc.sync.dma_start(out=bv_sb[:], in_=bvr[:])
                    nc.vector.memset(ones[:], 1.0)

                # PE warmup: dummy matmuls during the initial DMA wait so the
                # clock ramp (and HAM window) completes before the first real
                # matmul group arrives; fine-grained (N=128) so the filler
                # ends right as the first weight chunk lands (~5.2us).
                scr = sb.tile([64, 64 + P], F16, tag="scr", bufs=1, name="scr")
                nc.vector.memset(scr[:], 0.0)

                def warm(n):
                    for _ in range(n):
                        wp = psp.tile([64, P], F32, tag="mm", bufs=6, name="wp")
                        nc.tensor.matmul(wp[:], lhsT=scr[:, 0:64],
                                         rhs=scr[:, 64:64 + P],
                                         start=True, stop=True)

                if it in (None, 0):
                    # only the first unrolled body needs the ramp warmup; in
                    # steady state the previous body keeps the PE busy
                    warm(48)

                # block-diagonal kv operand; zeroed early, off the critical path
                kvbd = sb.tile([P, MT * P], F16, tag="kvr", bufs=1, name="kvbd")
                nc.vector.memset(kvbd[:], 0.0)

                # ---- Q projection groups: qt_all[:, g*NB:], g = nb*MT + m ----
                qt_all = sb.tile([P, 2 * MT * NB], F16, tag="qt", bufs=1,
                                 name="qt_all")

                def q_group(m, nb):
                    g = nb * MT + m
                    qp = psp.tile([P, NB], F32, tag="mm", bufs=6, name="qp")
                    for kt in range(KT):
                        nc.tensor.matmul(
                            qp[:],
                            lhsT=wqa[:, kt * C + m * P:kt * C + (m + 1) * P],
                            rhs=xTa[:, kt * SH + nb * NB:kt * SH + (nb + 1) * NB],
                            start=(kt == 0), stop=(kt == KT - 1),
                        )
                    nc.scalar.activation(qt_all[:, g * NB:(g + 1) * NB], qp[:],
                                         AF.Identity, bias=bq_sb[:, m:m + 1],
                                         scale=1.0)

                # ---- phase-1 helpers ----
                kv_ps_a = psp.tile([P, NB], F32, tag="acc", bufs=2, name="kv_ps_a")
                kv_ps_b = psp.tile([P, NB], F32, tag="acc", bufs=2, name="kv_ps_b")

                def proj_half(w_big, brow, st, nb, nm, fills=None):
                    t = sb.tile([P, NB], F16, tag="kvs", bufs=6, name=nm)
                    pk = psp.tile([P, NB], F32, tag="mm", bufs=6, name="pk")
                    for kt in range(KT):
                        if fills:
                            warm(fills.get(kt, 0))
                        nc.tensor.matmul(
                            pk[:],
                            lhsT=dTa[:, kt * SH + st * P:kt * SH + (st + 1) * P],
                            rhs=w_big[:, kt * C + nb * NB:kt * C + (nb + 1) * NB],
                            start=(kt == 0),
                            stop=(kt == KT - 1 and not with_bkv),
                        )
                    if with_bkv:
                        nc.tensor.matmul(
                            pk[:], lhsT=ones[:], rhs=brow[:, nb * NB:(nb + 1) * NB],
                            start=False, stop=True,
                        )
                    nc.scalar.copy(t[:], pk[:])
                    return t

                def kv_mms(k_h, v_h, st, half):
                    # head pairs half*4 .. half*4+3; pair tp at free offset
                    # (tp%4)*128 of bank a (tp<4) or b
                    for i in range(4):
                        tp = half * 4 + i
                        tgt = kv_ps_a if tp < 4 else kv_ps_b
                        fo = (tp % 4) * P
                        psl = slice(i * P, (i + 1) * P)
                        # start/stop once per PSUM bank (zero region), not
                        # per pair block
                        nc.tensor.matmul(
                            tgt[:, fo:fo + P],
                            lhsT=k_h[:, psl],
                            rhs=v_h[:, psl],
                            start=(st == 0 and i == 0),
                            stop=(st == MT - 1 and i == 3),
                        )

                kv_pend = []

                def p1_iter(st, nb, n_q, kf=None, vf=None):
                    # K group, then a lagged kv burst (2 iterations behind so
                    # the k/v evictions are never on the PE critical path),
                    # then V group, then any scheduled Q groups
                    k_h = proj_half(wka, bk_sb if with_bkv else None, st, nb,
                                    "k_h", kf)
                    if len(kv_pend) >= 2:
                        kv_mms(*kv_pend.pop(0))
                    v_h = proj_half(wva, bv_sb if with_bkv else None, st, nb,
                                    "v_h", vf)
                    kv_pend.append((k_h, v_h, st, nb))
                    for _ in range(n_q):
                        q_group(next(qiter), 0)

                qiter = iter(range(MT))

                # ---- kv eviction (diagonal quadrants) + pairwise AllReduce,
                # split per bank so each half reduces under ~30us of PE work --
                def kv_reduce(half, ps):
                    kv_s = sb.tile([P, NB // 2], F16, tag="kvio", bufs=2,
                                   name=f"kv_s{half}")
                    for r0 in (0, 64):
                        src = ps[r0:r0 + 64, :].rearrange(
                            "p (t x) -> p t x", x=P)[:, :, r0:r0 + 64]
                        dst = kv_s[r0:r0 + 64, :].rearrange(
                            "p (t x) -> p t x", x=64)
                        if r0 == 0:
                            nc.vector.tensor_copy(dst, src)
                        else:
                            nc.scalar.copy(dst, src)
                    kv_i = drp.tile([P, NB // 2], F16, tag="dri", bufs=2,
                                    name=f"kv_i{half}")
                    kv_o = drp.tile([P, NB // 2], F16, tag="dro", bufs=2,
                                    name=f"kv_o{half}")
                    nc.sync.dma_start(out=kv_i[:], in_=kv_s[:])
                    if n_cores == 1 or not use_cc:
                        # single-core analysis build: stand in for the AllReduce
                        nc.sync.dma_start(out=kv_o[:], in_=kv_i[:])
                    else:
                        nc.gpsimd.collective_compute(
                            "AllReduce", ALU.add, replica_groups=groups,
                            ins=[kv_i.opt()], outs=[kv_o.opt()],
                        )
                    # scatter diagonals into kvbd blocks half*4 .. half*4+3
                    for r0 in (0, 64):
                        dst = kvbd[r0:r0 + 64, half * 4 * P:(half + 1) * 4 * P
                                   ].rearrange("p (t x) -> p t x", x=P)[:, :,
                                                                        r0:r0 + 64]
                        src = kv_o[r0:r0 + 64, :].rearrange(
                            "p (t x) -> p t x", x=64)
                        nc.sync.dma_start(out=dst, in_=src)

                # ---- attention: one block-diagonal matmul per (m, nb) ----
                attn_sb = [sb.tile([P, SH], F16, tag="at", bufs=8, name=f"attn{m}")
                           for m in range(MT)]

                def attn_mm(m, nb):
                    g = nb * MT + m
                    nsl = slice(nb * NB, (nb + 1) * NB)
                    ap = psp.tile([P, NB], F32, tag="mm", bufs=6, name="ap")
                    nc.tensor.matmul(
                        ap[:],
                        lhsT=kvbd[:, m * P:(m + 1) * P],
                        rhs=qt_all[:, g * NB:(g + 1) * NB],
                        start=True, stop=True)
                    if m % 2 == 0:
                        nc.vector.tensor_copy(attn_sb[m][:, nsl], ap[:])
                    else:
                        nc.scalar.copy(attn_sb[m][:, nsl], ap[:])

                def mlp1_group(m, nb):
                    nsl = slice(nb * NB, (nb + 1) * NB)
                    hp = psp.tile([P, NB], F32, tag="mm", bufs=6, name="hp")
                    for kt in range(KT):
                        nc.tensor.matmul(
                            hp[:],
                            lhsT=w1a[:, kt * C + m * P:kt * C + (m + 1) * P],
                            rhs=attn_sb[kt][:, nsl],
                            start=(kt == 0), stop=(kt == KT - 1),
                        )
                    nc.scalar.activation(h1_sb[m][:, nsl], hp[:], AF.Gelu,
                                         bias=b1_sb[:, m:m + 1], scale=1.0)

                # ================= PE stream =================
                for st in range(MT):             # phase-1 nb=0 + Q nb=0 groups
                    p1_iter(st, 0, Q_SCHED[st])
                for m in qiter:                  # any Q groups Q_SCHED missed
                    q_group(m, 0)
                for st in range(MT):             # phase-1 nb=1
                    p1_iter(st, 1, 0)
                    if st == 1:                  # nb=0 kv bursts fully drained
                        kv_reduce(0, kv_ps_a)    # AR pairs 0-3 under nb=1 sweep

                if phases <= 1:
                    while kv_pend:
                        kv_mms(*kv_pend.pop(0))
                    kv_reduce(1, kv_ps_b)
                    # dump kvbd [128, 1024] into the first SH columns of outT
                    stg0 = sb.tile([P, MT * P], F16, tag="stgd", bufs=1, name="stg0")
                    nc.vector.tensor_copy(stg0[:], kvbd[:])
                    for m in range(MT):
                        nc.sync.dma_start(out=outT[m * P:(m + 1) * P, 0:MT * P],
                                          in_=stg0[:])
                    return

                # Q nb=1 sweep: drain the last kv bursts + kick AR-b early,
                # then attention nb=0 interleaved (m 0-3 need only AR half a;
                # m 4-7 sit as late as possible to maximize AR-b slack)
                attn0_sched = {2: [0], 3: [1], 4: [2], 5: [3],
                               6: [4], 7: [5, 6]}
                for m in range(MT):
                    q_group(m, 1)
                    while kv_pend:
                        kv_mms(*kv_pend.pop(0))
                        if not kv_pend:
                            kv_reduce(1, kv_ps_b)
                    for am in attn0_sched.get(m, []):
                        attn_mm(am, 0)

                if phases <= 2:
                    # only attn nb=0 exists at this phase cut; attn(7,0)
                    # normally sits in the MLP1 sweep, emit it here instead
                    attn_mm(7, 0)
                    for m in range(MT):
                        stg = sb.tile([P, NB], F16, tag="stg", bufs=3, name="stg")
                        nc.vector.tensor_copy(stg[:], attn_sb[m][:, 0:NB])
                        nc.sync.dma_start(out=outT[m * P:(m + 1) * P, 0:NB],
                                          in_=stg[:])
                    return

                # MLP1 nb=0 sweep with attention nb=1 interleaved; the last
                # attn nb=0 matmul (needs AR-b) goes first, right before the
                # MLP1 groups that consume it
                h1_sb = [sb.tile([P, SH], F16, tag="h1", bufs=8, name=f"h1{m}")
                         for m in range(MT)]
                attn_mm(0, 1)
                attn_mm(7, 0)
                attn_mm(1, 1)
                for m in range(MT):
                    mlp1_group(m, 0)
                    if m + 2 < MT:
                        attn_mm(m + 2, 1)
                for m in range(MT):
                    mlp1_group(m, 1)

                if phases <= 3:
                    for m in range(MT):
                        stg = sb.tile([P, SH], F16, tag="stg", bufs=3, name="stg")
                        nc.vector.tensor_copy(stg[:], h1_sb[m][:])
                        nc.sync.dma_start(out=outT[m * P:(m + 1) * P, :],
                                          in_=stg[:])
                    return

                # residual x + attn (f16; rel contribution ~2e-4)
                xacc = [sb.tile([P, SH], F16, tag="xr", bufs=8, name=f"xacc{m}")
                        for m in range(MT)]
                for m in range(MT):
                    nc.vector.tensor_add(
                        xacc[m][:],
                        xTa[:].rearrange("p (k s) -> p k s", k=KT)[:, m, :],
                        attn_sb[m][:])

                # ---- MLP out + residuals (m-outer; output DMA per half) ----
                for m in range(MT):
                    stg = sb.tile([P, SH], F16, tag="stg", bufs=3, name="stg")
                    for nb in range(NBLK):
                        nsl = slice(nb * NB, (nb + 1) * NB)
                        # the very last group runs as two N=256 psum groups so
                        # the final eviction+store chain is half as long
                        subs = ([(0, 256), (256, NB)]
                                if (m == MT - 1 and nb == NBLK - 1) else
                                [(0, NB)])
                        for s0, s1 in subs:
                            ssl = slice(nb * NB + s0, nb * NB + s1)
                            op = psp.tile([P, s1 - s0], F32, tag="mm", bufs=6,
                                          name="op")
                            for kt in range(KT):
                                nc.tensor.matmul(
                                    op[:],
                                    lhsT=w2a[:, kt * C + m * P:kt * C + (m + 1) * P],
                                    rhs=h1_sb[kt][:, ssl],
                                    start=(kt == 0), stop=(kt == KT - 1),
                                )
                            # stg = (op + b2) + (x + attn)
                            nc.vector.scalar_tensor_tensor(
                                stg[:, ssl], op[:], b2_sb[:, m:m + 1],
                                xacc[m][:, ssl], op0=ALU.add, op1=ALU.add)
                            nc.sync.dma_start(out=outT[m * P:(m + 1) * P, ssl],
                                              in_=stg[:, ssl])

            # Straight-line unroll for timing runs (collectives cannot sit
            # inside a hardware For_i loop on this execution path).
            for it_ in range(loop_r):
                body(it_)

    nc.compile()
    return nc


def _get_program(with_bkv: bool, loop_r: int = 1, use_cc: bool = True,
                 phases: int = 4):
    key = (with_bkv, loop_r, use_cc, phases)
    if key not in _CACHE:
        _CACHE[key] = _build(with_bkv, loop_r, use_cc=use_cc, phases=phases)
    return _CACHE[key]


def _pack_inputs(x, data, Wq, bq, Wk, bk, Wv, bv, W1, b1, W2, b2, with_bkv):
    f32, f16 = np.float32, np.float16
    wq_s = np.ascontiguousarray((np.asarray(Wq, f32) * f32(SCALE)).astype(f16))
    wk_c = np.ascontiguousarray(np.asarray(Wk, f32).astype(f16))
    wv_c = np.ascontiguousarray(np.asarray(Wv, f32).astype(f16))
    w1_c = np.ascontiguousarray(np.asarray(W1, f32).astype(f16))
    w2_c = np.ascontiguousarray(np.asarray(W2, f32).astype(f16))
    bqt = np.ascontiguousarray((np.asarray(bq, f32) * f32(SCALE)).reshape(MT, P).T)
    b1t = np.ascontiguousarray(np.asarray(b1, f32).reshape(MT, P).T)
    b2t = np.ascontiguousarray(np.asarray(b2, f32).reshape(MT, P).T)
    x = np.asarray(x, f32)
    data = np.asarray(data, f32)
    in_maps = []
    for c in range(8):
        b_, j = divmod(c, 2)
        m = {
            "xT": np.ascontiguousarray(x[b_, j * SH:(j + 1) * SH, :].T.astype(f16)),
            "dT": np.ascontiguousarray(data[b_, j * SH:(j + 1) * SH, :].T.astype(f16)),
            "wq": wq_s, "wk": wk_c, "wv": wv_c, "w1": w1_c, "w2": w2_c,
            "bqt": bqt, "b1t": b1t, "b2t": b2t,
        }
        if with_bkv:
            m["bkr"] = np.asarray(bk, f32).astype(f16).reshape(1, C)
            m["bvr"] = np.asarray(bv, f32).astype(f16).reshape(1, C)
        in_maps.append(m)
    return in_maps


def run_on_hw(inputs, loop_r: int = 1, trace: bool = False):
    """Run the SPMD program; returns BassKernelResults."""
    from concourse.bass_utils import run_bass_kernel_spmd

    with_bkv = bool(
        np.any(np.asarray(inputs["bk"])) or np.any(np.asarray(inputs["bv"]))
    )
    nc = _get_program(with_bkv, loop_r)
    in_maps = _pack_inputs(
        inputs["x"], inputs["data"], inputs["Wq"], inputs["bq"], inputs["Wk"],
        inputs["bk"], inputs["Wv"], inputs["bv"], inputs["W1"], inputs["b1"],
        inputs["W2"], inputs["b2"], with_bkv,
    )
    res = run_bass_kernel_spmd(nc, in_maps, list(range(8)), trace=trace)
    return res


def kernel(**inputs) -> np.ndarray:
    res = run_on_hw(inputs, loop_r=1)
    out = np.empty((B, S, C), dtype=np.float32)
    for c in range(8):
        b_, j = divmod(c, 2)
        out[b_, j * SH:(j + 1) * SH, :] = res.results[c]["outT"].astype(np.float32).T
    return out
